# revision 22
# baseline (speedup 1.0000x reference)
"""NeuralKB retrieval kernel v6: v4 steady state + split partition-reduce tail.

Sharding: 8 cores = 2 scores x 4 entity-quarters; per core B=8 batch slots,
NN=1024 entities (1000 real), F=4000 facts padded to 4096 (32 chunks of 128
facts on partitions).

Steady state (unchanged from v4, HW-balanced ACT/DVE at 4/4 slots):
per chunk, PE matmuls cp=2*ent@fc into PSUM; ACT produces slots 0-3
(activation bias-add straight from PSUM), DVE derives slots 4-7 from the
slot-0 carrier via tensor_scalar (4x mode), DVE folds the chunk into the
accumulator with one [128, 8192] bf16 tensor_tensor max (2x mode).

v6 change: the tail. v4 ended with one gpsimd partition_all_reduce over
[128, 8192] (~26us serial, HW-measured). v6 splits the last chunk's max into
per-slot TTs and then reduces each slot's [128, 1024] section over the
partition axis via two parallel paths: slots 0-3 on gpsimd
(partition_all_reduce, ~3.3us each, staggered), slots 4-7 via 8 PE
transposes + one DVE free-dim tensor_reduce each (PE is idle at that point).
Finals run in both layouts and DMA directly to the output rows.
"""

import contextlib

import numpy as np

import concourse.bass as bass
import concourse.tile as tile
from concourse import bacc, mybir
from concourse import bass_utils
from concourse.masks import make_identity
from concourse.bass_isa import ReduceOp

F32 = mybir.dt.float32
BF16 = mybir.dt.bfloat16
AF = mybir.ActivationFunctionType
ALU = mybir.AluOpType

B = 8
E = 100
F = 4000
FP = 4096
NCHUNK = FP // 128
GROUPS = 4
GCH = NCHUNK // GROUPS  # 8
NN = 1000
NHALF = 512          # PSUM-bank-aligned first half; second half is 488 cols
XW = B * NN

DVE_SLOTS = (4, 5, 6, 7)
ACT_SLOTS = (1, 2, 3)

NGP = 4          # tail slots reduced on gpsimd; the rest via PE transposes
# chunks where slot 4 is produced by ACT instead of DVE (DVE is the pole;
# these chunks carry no prologue pieces so ACT has local slack)
SWAP_ACT_CHUNKS = (10, 13, 16, 21, 24, 26, 28)
NB = 8           # transpose blocks per slot
NBW = NN // NB   # 125


def build_bass(repeat=1, dve_slots=DVE_SLOTS, act_slots=ACT_SLOTS,
               debug=False, scope="full"):
    assert sorted((0,) + tuple(dve_slots) + tuple(act_slots)) == list(range(B))
    nc = bacc.Bacc("TRN2", target_bir_lowering=False, debug=False, num_devices=8)

    f_c = nc.dram_tensor("f_c", [FP, E], F32, kind="ExternalInput")
    f_w = nc.dram_tensor("f_w", [FP, E], F32, kind="ExternalInput")
    f_rel = nc.dram_tensor("f_rel", [FP, E], F32, kind="ExternalInput")
    ent = nc.dram_tensor("ent", [1024, E], F32, kind="ExternalInput")
    rel = nc.dram_tensor("rel", [B, E], F32, kind="ExternalInput")
    qw = nc.dram_tensor("qw", [B, E], F32, kind="ExternalInput")
    out = nc.dram_tensor("out", [B, 1024], F32, kind="ExternalOutput")

    with tile.TileContext(nc) as tc:
        with (
            tc.tile_pool(name="const", bufs=1) as const_pool,
            tc.tile_pool(name="factT", bufs=1) as factT_pool,
            tc.tile_pool(name="acc", bufs=1) as acc_pool,
            tc.tile_pool(name="small", bufs=1) as small_pool,
            tc.tile_pool(name="nat", bufs=1) as nat_pool,
            tc.tile_pool(name="sq", bufs=2) as sq_pool,
            tc.tile_pool(name="xall", bufs=3) as xall_pool,
            tc.tile_pool(name="fin", bufs=1) as fin_pool,
            tc.tile_pool(name="tpsum", bufs=2, space="PSUM") as tpsum_pool,
            tc.tile_pool(name="cpsum", bufs=2, space="PSUM") as cpsum_pool,
            tc.tile_pool(name="wpsum", bufs=1, space="PSUM") as wpsum_pool,
        ):
            pools = (const_pool, factT_pool, acc_pool, small_pool, nat_pool,
                     sq_pool, xall_pool, fin_pool, tpsum_pool, cpsum_pool,
                     wpsum_pool)

            ident = const_pool.tile([128, 128], F32, tag="ident")
            make_identity(nc, ident[:])
            identb = const_pool.tile([128, 128], BF16, tag="identb")
            nc.scalar.activation(identb[:], ident[:], AF.Copy)
            frelT = factT_pool.tile([101, FP], BF16, tag="frelT")
            fwT = factT_pool.tile([101, FP], BF16, tag="fwT")
            fcT = factT_pool.tile([100, FP], BF16, tag="fcT")
            onesrow = small_pool.tile([1, FP], BF16, tag="onesrow")
            nc.gpsimd.memset(onesrow[:], 1.0)
            nc.sync.dma_start(frelT[100:101, :], onesrow[:])
            relmov = const_pool.tile([101, B], BF16, tag="relmov")
            qwmov = const_pool.tile([101, B], BF16, tag="qwmov")
            negrow = small_pool.tile([1, B], BF16, tag="negrow")
            nc.gpsimd.memset(negrow[:], -1.0)
            ones_col = const_pool.tile([100, 1], BF16, tag="ones_col")
            nc.gpsimd.memset(ones_col[:], 1.0)
            statics = (ident, identb, frelT, fwT, fcT, relmov, qwmov, ones_col, negrow)

            rep_ctx = tc.For_i(0, repeat, 1) if repeat > 1 else contextlib.nullcontext()
            with rep_ctx:
                _full_body(nc, tc, pools, statics, f_c, f_w, f_rel, ent,
                           rel, qw, out, dve_slots, act_slots)
    nc.compile()
    return nc


def _full_body(nc, tc, pools, statics, f_c, f_w, f_rel, ent, rel, qw, out,
               dve_slots, act_slots):
    (const_pool, factT_pool, acc_pool, small_pool, nat_pool, sq_pool,
     xall_pool, fin_pool, tpsum_pool, cpsum_pool, wpsum_pool) = pools
    (ident, identb, frelT, fwT, fcT, relmov, qwmov, ones_col, negrow) = statics

    # ---------------- input loads (parallel DGE queues) ----------------------
    nats = []
    for i, dram in enumerate((f_rel, f_w, f_c)):
        natt = nat_pool.tile([128, NCHUNK * E], F32, tag=f"nat_{i}")
        nats.append(natt)

    def load_chunks(c0, c1, engines):
        for i, dram in enumerate((f_rel, f_w, f_c)):
            engines[i].dma_start(
                nats[i][:, c0 * E : c1 * E].rearrange("p (c e) -> p c e", e=E),
                dram.ap()[c0 * 128 : c1 * 128, :].rearrange(
                    "(c p) e -> p c e", p=128
                ),
            )

    qts = {}
    for name, dram in (("rel", rel), ("qw", qw)):
        qt = small_pool.tile([B, E], F32, tag=f"q_{name}")
        nc.sync.dma_start(qt[:], dram.ap())
        qts[name] = qt
    # group-0 chunks first (the W/f2 prologue and first carriers need them,
    # and their consumer chain is longer than the entity path's);
    # alternate the two HWDGE queues per batch to balance bytes.
    load_chunks(0, 4, (nc.sync, nc.scalar, nc.sync))
    entn = nat_pool.tile([128, 8 * E], F32, tag="entn")
    nc.sync.dma_start(
        entn[:].rearrange("p (c e) -> p c e", e=E),
        ent.ap().rearrange("(c p) e -> p c e", p=128),
    )
    load_chunks(4, 8, (nc.scalar, nc.sync, nc.scalar))
    load_chunks(8, 20, (nc.scalar, nc.sync, nc.scalar))
    load_chunks(20, 32, (nc.sync, nc.scalar, nc.sync))

    # ---------------- queries ------------------------------------------------
    for name, dst in (("rel", relmov), ("qw", qwmov)):
        tp = tpsum_pool.tile([128, NHALF], F32, tag="tp")
        nc.tensor.transpose(tp[:E, :B], qts[name][:], ident[:B, :B])
        nc.scalar.activation(dst[0:100, :], tp[:E, :B], AF.Copy, scale=2.0)
    sqs = {}
    for name in ("rel", "qw"):
        sq = small_pool.tile([B, E], F32, tag=f"qsq_{name}")
        nc.scalar.activation(sq[:], qts[name][:], AF.Square)
        r = small_pool.tile([B, 1], F32, tag=f"qr_{name}")
        nc.vector.tensor_reduce(r[:], sq[:], axis=mybir.AxisListType.X, op=ALU.add)
        sqs[name] = r
    q2 = small_pool.tile([B, 1], F32, tag="q2")
    nc.vector.tensor_tensor(q2[:], sqs["rel"][:], sqs["qw"][:], op=ALU.add)
    q2tp = tpsum_pool.tile([128, NHALF], F32, tag="tp")
    nc.tensor.transpose(q2tp[:1, :B], q2[:], ident[:B, :B])
    q2neg = small_pool.tile([1, B], BF16, tag="q2neg")
    nc.scalar.activation(q2neg[:], q2tp[:1, :B], AF.Copy, scale=-1.0)
    nc.sync.dma_start(relmov[100:101, :], q2neg[:])

    # ---------------- entities ----------------------------------------------
    ent2T = const_pool.tile([100, 1024], BF16, tag="ent2T")
    for c in range(8):
        tp = tpsum_pool.tile([128, NHALF], F32, tag="tp")
        nc.tensor.transpose(tp[:100, :128], entn[:, c * E : (c + 1) * E], ident[:])
        nc.vector.tensor_scalar(
            out=ent2T[:, c * 128 : (c + 1) * 128], in0=tp[:100, :128],
            scalar1=2.0, scalar2=None, op0=ALU.mult)
    e2row = small_pool.tile([1, NN], F32, tag="e2row")
    e2k = fin_pool.tile([NBW, NB], F32, tag="e2k")
    e2rep = fin_pool.tile([NGP, NN], F32, tag="e2rep")

    def emit_e2():
        entsqT = sq_pool.tile([100, NN], BF16, tag="entsqT")
        nc.vector.tensor_tensor(entsqT[:], ent2T[:, 0:NN], ent2T[:, 0:NN],
                                op=ALU.mult)
        for h0, h1 in ((0, NHALF), (NHALF, NN)):
            e2p = tpsum_pool.tile([128, NHALF], F32, tag="tp")
            nc.tensor.matmul(e2p[:1, 0 : h1 - h0], ones_col[:],
                             entsqT[:, h0:h1], start=True, stop=True)
            nc.scalar.activation(e2row[:, h0:h1], e2p[:1, 0 : h1 - h0],
                                 AF.Copy, scale=0.25)
        # e2 in [NBW, NB] layout (n = k*NBW + p) for the transpose-path finals
        e2tp = tpsum_pool.tile([128, NHALF], F32, tag="tp")
        for k in range(NB):
            nc.tensor.transpose(e2tp[:NBW, k * 4 : k * 4 + 1],
                                e2row[0:1, k * NBW : (k + 1) * NBW],
                                ident[:1, :1])
        nc.vector.tensor_copy(
            e2k[:, :].rearrange("p (k o) -> p k o", o=1),
            e2tp[:NBW, :].rearrange("p (k x) -> p k x", x=4)[:, 0:NB, 0:1])
        nc.gpsimd.partition_broadcast(e2rep[:], e2row[:])

    wpsum = wpsum_pool.tile([128, 512], F32, tag="wpsum")
    W_sb = const_pool.tile([128, NCHUNK * B], F32, tag="W_sb")
    Weff = const_pool.tile([128, NCHUNK * B], F32, tag="Weff")
    acc_all = acc_pool.tile([128, XW], BF16, tag="acc_all")

    # ---------------- pipelined prologue pieces + stage-1 --------------------
    def tcasts_quad(c0):
        """PE transposes for chunks c0..c0+3 + one wide cast per tensor."""
        for i, dstT in enumerate((frelT, fwT, fcT)):
            tp = tpsum_pool.tile([128, NHALF], F32, tag="tp")
            for k in range(4):
                c = c0 + k
                ces = slice(c * E, (c + 1) * E)
                nc.tensor.transpose(tp[:100, k * 128 : (k + 1) * 128],
                                    nats[i][:, ces], ident[:])
            if (c0 < 16 and i == 0) or (c0 < 8 and i == 1):
                nc.vector.tensor_scalar(
                    out=dstT[0:100, c0 * 128 : (c0 + 4) * 128],
                    in0=tp[:100, :], scalar1=1.0, scalar2=None, op0=ALU.mult)
            else:
                nc.scalar.activation(
                    dstT[0:100, c0 * 128 : (c0 + 4) * 128], tp[:100, :],
                    AF.Copy)

    def group_fw(g):
        """f2 + W columns for the 8 chunks of group g."""
        gs = slice(g * GCH * 128, (g + 1) * GCH * 128)  # 1024 f cols
        sqg = sq_pool.tile([100, 3 * 1024], BF16, tag="sqg")
        for i, srcT in enumerate((frelT, fwT, fcT)):
            if g == 0:
                # DVE is idle during the ramp; keep ACT off the critical path
                nc.vector.tensor_tensor(sqg[:, i * 1024 : (i + 1) * 1024],
                                        srcT[0:100, gs], srcT[0:100, gs],
                                        op=ALU.mult)
            else:
                nc.scalar.activation(sqg[:, i * 1024 : (i + 1) * 1024],
                                     srcT[0:100, gs], AF.Square)
        f2st = sq_pool.tile([1, 1024], BF16, tag="f2st")
        for h in range(2):
            f2p = tpsum_pool.tile([128, NHALF], F32, tag="tp")
            for i in range(3):
                nc.tensor.matmul(
                    f2p[:1, 0:512], ones_col[:],
                    sqg[:, i * 1024 + h * 512 : i * 1024 + (h + 1) * 512],
                    start=(i == 0), stop=(i == 2))
            nc.scalar.activation(f2st[:, h * 512 : (h + 1) * 512],
                                 f2p[:1, 0:512], AF.Copy)
        for c in range(g * GCH, (g + 1) * GCH):
            cs = slice(c * 128, (c + 1) * 128)
            ws = slice(c * B, (c + 1) * B)
            lc = c - g * GCH
            nc.tensor.matmul(wpsum[:, ws], frelT[:, cs], relmov[:],
                             start=True, stop=False)
            nc.tensor.matmul(wpsum[:, ws], fwT[0:100, cs], qwmov[0:100, :],
                             start=False, stop=False)
            nc.tensor.matmul(wpsum[:, ws],
                             f2st[0:1, lc * 128 : (lc + 1) * 128],
                             negrow[:], start=False, stop=True)
        gws = slice(g * GCH * B, (g + 1) * GCH * B)
        nc.scalar.activation(W_sb[:, gws], wpsum[:, gws], AF.Copy)
        wv = W_sb[:, gws].rearrange("p (c s) -> p c s", s=B)
        ev = Weff[:, gws].rearrange("p (c s) -> p c s", s=B)
        nc.vector.tensor_tensor(
            ev[:, :, 1:B], wv[:, :, 1:B],
            wv[:, :, 0:1].broadcast_to([128, GCH, B - 1]), op=ALU.subtract)

    def group_fw_half(g, h):
        """f2 + W columns for half h (4 chunks) of group g (ramp only)."""
        c0 = g * GCH + h * 4
        gs = slice(c0 * 128, (c0 + 4) * 128)  # 512 f cols
        sqg = sq_pool.tile([100, 3 * 512], BF16, tag="sqgh")
        for i, srcT in enumerate((frelT, fwT, fcT)):
            nc.vector.tensor_tensor(sqg[:, i * 512 : (i + 1) * 512],
                                    srcT[0:100, gs], srcT[0:100, gs],
                                    op=ALU.mult)
        f2st = sq_pool.tile([1, 512], BF16, tag="f2sth")
        f2p = tpsum_pool.tile([128, NHALF], F32, tag="tp")
        for i in range(3):
            nc.tensor.matmul(f2p[:1, 0:512], ones_col[:],
                             sqg[:, i * 512 : (i + 1) * 512],
                             start=(i == 0), stop=(i == 2))
        nc.scalar.activation(f2st[:], f2p[:1, 0:512], AF.Copy)
        for c in range(c0, c0 + 4):
            cs = slice(c * 128, (c + 1) * 128)
            ws = slice(c * B, (c + 1) * B)
            lc = c - c0
            nc.tensor.matmul(wpsum[:, ws], frelT[:, cs], relmov[:],
                             start=True, stop=False)
            nc.tensor.matmul(wpsum[:, ws], fwT[0:100, cs], qwmov[0:100, :],
                             start=False, stop=False)
            nc.tensor.matmul(wpsum[:, ws],
                             f2st[0:1, lc * 128 : (lc + 1) * 128],
                             negrow[:], start=False, stop=True)
        gws = slice(c0 * B, (c0 + 4) * B)
        nc.scalar.activation(W_sb[:, gws], wpsum[:, gws], AF.Copy)
        wv = W_sb[:, gws].rearrange("p (c s) -> p c s", s=B)
        ev = Weff[:, gws].rearrange("p (c s) -> p c s", s=B)
        nc.vector.tensor_tensor(
            ev[:, :, 1:B], wv[:, :, 1:B],
            wv[:, :, 0:1].broadcast_to([128, 4, B - 1]), op=ALU.subtract)

    # ---------------- tail: per-slot partition reduction ---------------------
    gpred = [None] * NGP
    m5 = fin_pool.tile([NBW, (B - NGP) * NB], F32, tag="m5")

    def _tail_reduce(s):
        sec = slice(s * NN, (s + 1) * NN)
        if s < NGP:
            red = fin_pool.tile([128, NN], BF16, tag=f"gpred{s}")
            nc.gpsimd.partition_all_reduce(red[:], acc_all[:, sec], 128,
                                           ReduceOp.max)
            gpred[s] = red
        else:
            ttp = wpsum_pool.tile([128, NB * 128], BF16, tag="ttp")
            for kk in range(NB):
                bs = slice(s * NN + kk * NBW, s * NN + (kk + 1) * NBW)
                nc.tensor.transpose(ttp[:NBW, kk * 128 : (kk + 1) * 128],
                                    acc_all[:, bs], identb[:])
            mcol = (s - NGP) * NB
            nc.vector.tensor_reduce(
                m5[:, mcol : mcol + NB],
                ttp[:NBW, :].rearrange("p (k x) -> p k x", x=128),
                axis=mybir.AxisListType.X, op=ALU.max)

    def stage1(c, tail=False):
        cs = slice(c * 128, (c + 1) * 128)
        wcol = lambda s: W_sb[:, c * B + s : c * B + s + 1]
        dcol = lambda s: Weff[:, c * B + s : c * B + s + 1]
        cp = cpsum_pool.tile([128, NN], F32, tag="cp")
        nc.tensor.matmul(cp[:, 0:NHALF], fcT[0:100, cs], ent2T[:, 0:NHALF],
                         start=True, stop=True)
        # second half is 488 cols: bytes 2048..4000 stay inside PSUM bank 1
        nc.tensor.matmul(cp[:, NHALF:NN], fcT[0:100, cs], ent2T[:, NHALF:NN],
                         start=True, stop=True)
        xt = acc_all if c == 0 else xall_pool.tile([128, XW], BF16, tag="xall")
        xsec = lambda s: xt[:, s * NN : (s + 1) * NN]
        carrier = xsec(0)
        nc.scalar.activation(carrier, cp[:], AF.Identity, bias=wcol(0))
        extra = (4,) if c in SWAP_ACT_CHUNKS else ()
        for s in tuple(act_slots) + extra:
            nc.scalar.activation(xsec(s), cp[:], AF.Identity, bias=wcol(s))
        for s in (x for x in dve_slots if x not in extra):
            nc.vector.tensor_scalar(
                out=xsec(s), in0=carrier, scalar1=dcol(s), scalar2=None,
                op0=ALU.add)
        if c == 0:
            return
        if not tail:
            nc.vector.tensor_tensor(acc_all[:], acc_all[:], xt[:], op=ALU.max)
        else:
            for s in range(B):
                sec = slice(s * NN, (s + 1) * NN)
                nc.vector.tensor_tensor(acc_all[:, sec], acc_all[:, sec],
                                        xt[:, sec], op=ALU.max)
                _tail_reduce(s)

    # ---------------- main loop ----------------------------------------------
    tcasts_quad(0)
    group_fw_half(0, 0)
    stage1(0)
    stage1(1)
    tcasts_quad(4)
    group_fw_half(0, 1)
    stage1(2)
    stage1(3)
    for g in range(GROUPS):
        for ci in range(4 if g == 0 else 0, GCH):
            c = g * GCH + ci
            stage1(c, tail=(c == NCHUNK - 1))
            if c == 18:
                emit_e2()
            if g + 1 < GROUPS:
                if ci == (4 if g == 0 else 1):
                    tcasts_quad((g + 1) * GCH)
                elif ci == (5 if g == 0 else 4):
                    tcasts_quad((g + 1) * GCH + 4)
                elif ci == GCH - 1:
                    group_fw(g + 1)

    # ---------------- finals -------------------------------------------------
    # gpsimd-path slots (0..NGP-1): row layout [NGP, NN]
    mh = fin_pool.tile([NGP, NN], BF16, tag="mh")
    for s in range(NGP):
        eng = nc.sync if s % 2 == 0 else nc.scalar
        eng.dma_start(mh[s : s + 1, :], gpred[s][0:1, :])
    subh = fin_pool.tile([NGP, NN], F32, tag="subh")
    nc.vector.tensor_tensor(subh[:], e2rep[:], mh[:], op=ALU.subtract)
    nc.scalar.activation(subh[:], subh[:], AF.Exp, scale=-0.5)
    nc.vector.tensor_scalar(
        out=subh[:], in0=subh[:], scalar1=1.0, scalar2=None, op0=ALU.min)
    nc.sync.dma_start(out.ap()[0:NGP, 0:NN], subh[:])

    # transpose-path slots (NGP..B-1): m5 [NBW, (B-NGP)*NB], n = k*NBW + p
    fin5 = fin_pool.tile([NBW, (B - NGP) * NB], F32, tag="fin5")
    for j in range(B - NGP):
        js = slice(j * NB, (j + 1) * NB)
        nc.vector.tensor_tensor(fin5[:, js], e2k[:, :], m5[:, js],
                                op=ALU.subtract)
    nc.scalar.activation(fin5[:], fin5[:], AF.Exp, scale=-0.5)
    nc.vector.tensor_scalar(
        out=fin5[:], in0=fin5[:], scalar1=1.0, scalar2=None, op0=ALU.min)
    for j in range(B - NGP):
        eng = nc.sync if j % 2 == 0 else nc.scalar
        eng.dma_start(
            out.ap()[NGP + j : NGP + j + 1, 0:NN].rearrange(
                "o (k p) -> (o p) k", p=NBW),
            fin5[:, j * NB : (j + 1) * NB],
        )


_NC_CACHE = None


def get_nc():
    global _NC_CACHE
    if _NC_CACHE is None:
        _NC_CACHE = build_bass()
    return _NC_CACHE


def make_in_maps(rel, arg1, arg2, fact_rel, fact_arg1, fact_arg2, entity_embeddings):
    n_per = F // 4

    def pad_fact(m):
        o = np.full((FP, E), 10.0, dtype=np.float32)
        o[:F] = m
        return o

    frp = pad_fact(fact_rel)
    f1p = pad_fact(fact_arg1)
    f2p = pad_fact(fact_arg2)
    relc = np.ascontiguousarray(rel, dtype=np.float32)
    a1c = np.ascontiguousarray(arg1, dtype=np.float32)
    a2c = np.ascontiguousarray(arg2, dtype=np.float32)
    in_maps = []
    for core in range(8):
        score, slot = (0, core) if core < 4 else (1, core - 4)
        ent_pad = np.zeros((1024, E), dtype=np.float32)
        ent_pad[:n_per] = entity_embeddings[slot * n_per : (slot + 1) * n_per]
        if score == 0:
            fc_, fw_, qw_ = f2p, f1p, a1c
        else:
            fc_, fw_, qw_ = f1p, f2p, a2c
        in_maps.append(
            {"f_c": fc_, "f_w": fw_, "f_rel": frp, "ent": ent_pad,
             "rel": relc, "qw": qw_}
        )
    return in_maps


def assemble(results):
    n_per = F // 4
    sp = np.concatenate([results[i]["out"][:, :n_per] for i in range(4)], axis=1)
    po = np.concatenate([results[i]["out"][:, :n_per] for i in range(4, 8)], axis=1)
    return sp.copy(), po.copy()


def kernel(rel, arg1, arg2, fact_rel, fact_arg1, fact_arg2, entity_embeddings):
    nc = get_nc()
    in_maps = make_in_maps(
        rel, arg1, arg2, fact_rel, fact_arg1, fact_arg2, entity_embeddings
    )
    res = bass_utils.run_bass_kernel_spmd(nc, in_maps, core_ids=list(range(8)))
    return assemble(res.results)


# revision 23
# speedup vs baseline: 1.0288x; 1.0288x over previous
"""NeuralKB retrieval kernel v6: v4 steady state + split partition-reduce tail.

Sharding: 8 cores = 2 scores x 4 entity-quarters; per core B=8 batch slots,
NN=1024 entities (1000 real), F=4000 facts padded to 4096 (32 chunks of 128
facts on partitions).

Steady state (unchanged from v4, HW-balanced ACT/DVE at 4/4 slots):
per chunk, PE matmuls cp=2*ent@fc into PSUM; ACT produces slots 0-3
(activation bias-add straight from PSUM), DVE derives slots 4-7 from the
slot-0 carrier via tensor_scalar (4x mode), DVE folds the chunk into the
accumulator with one [128, 8192] bf16 tensor_tensor max (2x mode).

v6 change: the tail. v4 ended with one gpsimd partition_all_reduce over
[128, 8192] (~26us serial, HW-measured). v6 splits the last chunk's max into
per-slot TTs and then reduces each slot's [128, 1024] section over the
partition axis via two parallel paths: slots 0-3 on gpsimd
(partition_all_reduce, ~3.3us each, staggered), slots 4-7 via 8 PE
transposes + one DVE free-dim tensor_reduce each (PE is idle at that point).
Finals run in both layouts and DMA directly to the output rows.
"""

import contextlib

import numpy as np

import concourse.bass as bass
import concourse.tile as tile
from concourse import bacc, mybir
from concourse import bass_utils
from concourse.masks import make_identity
from concourse.bass_isa import ReduceOp

F32 = mybir.dt.float32
BF16 = mybir.dt.bfloat16
AF = mybir.ActivationFunctionType
ALU = mybir.AluOpType

B = 8
E = 100
F = 4000
FP = 4096
NCHUNK = FP // 128
GROUPS = 4
GCH = NCHUNK // GROUPS  # 8
NN = 1000
NHALF = 512          # PSUM-bank-aligned first half; second half is 488 cols
XW = B * NN

DVE_SLOTS = (4, 5, 6, 7)
ACT_SLOTS = (1, 2, 3)

NGP = 4          # tail slots reduced on gpsimd; the rest via PE transposes
# chunks where slot 4 is produced by ACT instead of DVE (DVE is the pole;
# these chunks carry no prologue pieces so ACT has local slack)
SWAP_ACT_CHUNKS = (10, 13, 16, 21, 24, 26, 28)
NB = 8           # transpose blocks per slot
NBW = NN // NB   # 125


def build_bass(repeat=1, dve_slots=DVE_SLOTS, act_slots=ACT_SLOTS,
               debug=False, scope="full"):
    assert sorted((0,) + tuple(dve_slots) + tuple(act_slots)) == list(range(B))
    nc = bacc.Bacc("TRN2", target_bir_lowering=False, debug=False, num_devices=8)

    f_c = nc.dram_tensor("f_c", [FP, E], F32, kind="ExternalInput")
    f_w = nc.dram_tensor("f_w", [FP, E], F32, kind="ExternalInput")
    f_rel = nc.dram_tensor("f_rel", [FP, E], F32, kind="ExternalInput")
    ent = nc.dram_tensor("ent", [1024, E], F32, kind="ExternalInput")
    rel = nc.dram_tensor("rel", [B, E], F32, kind="ExternalInput")
    qw = nc.dram_tensor("qw", [B, E], F32, kind="ExternalInput")
    out = nc.dram_tensor("out", [B, 1024], F32, kind="ExternalOutput")

    with tile.TileContext(nc) as tc:
        with (
            tc.tile_pool(name="const", bufs=1) as const_pool,
            tc.tile_pool(name="factT", bufs=1) as factT_pool,
            tc.tile_pool(name="acc", bufs=1) as acc_pool,
            tc.tile_pool(name="small", bufs=1) as small_pool,
            tc.tile_pool(name="nat", bufs=1) as nat_pool,
            tc.tile_pool(name="sq", bufs=2) as sq_pool,
            tc.tile_pool(name="xall", bufs=4) as xall_pool,
            tc.tile_pool(name="fin", bufs=1) as fin_pool,
            tc.tile_pool(name="tpsum", bufs=2, space="PSUM") as tpsum_pool,
            tc.tile_pool(name="cpsum", bufs=2, space="PSUM") as cpsum_pool,
            tc.tile_pool(name="wpsum", bufs=1, space="PSUM") as wpsum_pool,
        ):
            pools = (const_pool, factT_pool, acc_pool, small_pool, nat_pool,
                     sq_pool, xall_pool, fin_pool, tpsum_pool, cpsum_pool,
                     wpsum_pool)

            ident = const_pool.tile([128, 128], F32, tag="ident")
            make_identity(nc, ident[:])
            identb = const_pool.tile([128, 128], BF16, tag="identb")
            nc.scalar.activation(identb[:], ident[:], AF.Copy)
            frelT = factT_pool.tile([101, FP], BF16, tag="frelT")
            fwT = factT_pool.tile([101, FP], BF16, tag="fwT")
            fcT = factT_pool.tile([100, FP], BF16, tag="fcT")
            onesrow = small_pool.tile([1, FP], BF16, tag="onesrow")
            nc.gpsimd.memset(onesrow[:], 1.0)
            nc.sync.dma_start(frelT[100:101, :], onesrow[:])
            relmov = const_pool.tile([101, B], BF16, tag="relmov")
            qwmov = const_pool.tile([101, B], BF16, tag="qwmov")
            negrow = small_pool.tile([1, B], BF16, tag="negrow")
            nc.gpsimd.memset(negrow[:], -1.0)
            ones_col = const_pool.tile([100, 1], BF16, tag="ones_col")
            nc.gpsimd.memset(ones_col[:], 1.0)
            statics = (ident, identb, frelT, fwT, fcT, relmov, qwmov, ones_col, negrow)

            rep_ctx = tc.For_i(0, repeat, 1) if repeat > 1 else contextlib.nullcontext()
            with rep_ctx:
                _full_body(nc, tc, pools, statics, f_c, f_w, f_rel, ent,
                           rel, qw, out, dve_slots, act_slots)
    nc.compile()
    return nc


def _full_body(nc, tc, pools, statics, f_c, f_w, f_rel, ent, rel, qw, out,
               dve_slots, act_slots):
    (const_pool, factT_pool, acc_pool, small_pool, nat_pool, sq_pool,
     xall_pool, fin_pool, tpsum_pool, cpsum_pool, wpsum_pool) = pools
    (ident, identb, frelT, fwT, fcT, relmov, qwmov, ones_col, negrow) = statics

    # ---------------- input loads (parallel DGE queues) ----------------------
    nats = []
    for i, dram in enumerate((f_rel, f_w, f_c)):
        natt = nat_pool.tile([128, NCHUNK * E], F32, tag=f"nat_{i}")
        nats.append(natt)

    def load_chunks(c0, c1, engines):
        for i, dram in enumerate((f_rel, f_w, f_c)):
            engines[i].dma_start(
                nats[i][:, c0 * E : c1 * E].rearrange("p (c e) -> p c e", e=E),
                dram.ap()[c0 * 128 : c1 * 128, :].rearrange(
                    "(c p) e -> p c e", p=128
                ),
            )

    qts = {}
    for name, dram in (("rel", rel), ("qw", qw)):
        qt = small_pool.tile([B, E], F32, tag=f"q_{name}")
        nc.sync.dma_start(qt[:], dram.ap())
        qts[name] = qt
    # group-0 chunks first (the W/f2 prologue and first carriers need them,
    # and their consumer chain is longer than the entity path's);
    # alternate the two HWDGE queues per batch to balance bytes.
    load_chunks(0, 4, (nc.sync, nc.scalar, nc.sync))
    entn = nat_pool.tile([128, 8 * E], F32, tag="entn")
    nc.sync.dma_start(
        entn[:].rearrange("p (c e) -> p c e", e=E),
        ent.ap().rearrange("(c p) e -> p c e", p=128),
    )
    load_chunks(4, 8, (nc.scalar, nc.sync, nc.scalar))
    load_chunks(8, 20, (nc.scalar, nc.sync, nc.scalar))
    load_chunks(20, 32, (nc.sync, nc.scalar, nc.sync))

    # ---------------- queries ------------------------------------------------
    for name, dst in (("rel", relmov), ("qw", qwmov)):
        tp = tpsum_pool.tile([128, NHALF], F32, tag="tp")
        nc.tensor.transpose(tp[:E, :B], qts[name][:], ident[:B, :B])
        nc.scalar.activation(dst[0:100, :], tp[:E, :B], AF.Copy, scale=2.0)
    sqs = {}
    for name in ("rel", "qw"):
        sq = small_pool.tile([B, E], F32, tag=f"qsq_{name}")
        nc.scalar.activation(sq[:], qts[name][:], AF.Square)
        r = small_pool.tile([B, 1], F32, tag=f"qr_{name}")
        nc.vector.tensor_reduce(r[:], sq[:], axis=mybir.AxisListType.X, op=ALU.add)
        sqs[name] = r
    q2 = small_pool.tile([B, 1], F32, tag="q2")
    nc.vector.tensor_tensor(q2[:], sqs["rel"][:], sqs["qw"][:], op=ALU.add)
    q2tp = tpsum_pool.tile([128, NHALF], F32, tag="tp")
    nc.tensor.transpose(q2tp[:1, :B], q2[:], ident[:B, :B])
    q2neg = small_pool.tile([1, B], BF16, tag="q2neg")
    nc.scalar.activation(q2neg[:], q2tp[:1, :B], AF.Copy, scale=-1.0)
    nc.sync.dma_start(relmov[100:101, :], q2neg[:])

    # ---------------- entities ----------------------------------------------
    ent2T = const_pool.tile([100, 1024], BF16, tag="ent2T")
    for c in range(8):
        tp = tpsum_pool.tile([128, NHALF], F32, tag="tp")
        nc.tensor.transpose(tp[:100, :128], entn[:, c * E : (c + 1) * E], ident[:])
        nc.vector.tensor_scalar(
            out=ent2T[:, c * 128 : (c + 1) * 128], in0=tp[:100, :128],
            scalar1=2.0, scalar2=None, op0=ALU.mult)
    e2row = small_pool.tile([1, NN], F32, tag="e2row")
    e2k = fin_pool.tile([NBW, NB], F32, tag="e2k")
    e2rep = fin_pool.tile([NGP, NN], F32, tag="e2rep")

    def emit_e2():
        entsqT = sq_pool.tile([100, NN], BF16, tag="entsqT")
        nc.vector.tensor_tensor(entsqT[:], ent2T[:, 0:NN], ent2T[:, 0:NN],
                                op=ALU.mult)
        for h0, h1 in ((0, NHALF), (NHALF, NN)):
            e2p = tpsum_pool.tile([128, NHALF], F32, tag="tp")
            nc.tensor.matmul(e2p[:1, 0 : h1 - h0], ones_col[:],
                             entsqT[:, h0:h1], start=True, stop=True)
            nc.scalar.activation(e2row[:, h0:h1], e2p[:1, 0 : h1 - h0],
                                 AF.Copy, scale=0.25)
        # e2 in [NBW, NB] layout (n = k*NBW + p) for the transpose-path finals
        e2tp = tpsum_pool.tile([128, NHALF], F32, tag="tp")
        for k in range(NB):
            nc.tensor.transpose(e2tp[:NBW, k * 4 : k * 4 + 1],
                                e2row[0:1, k * NBW : (k + 1) * NBW],
                                ident[:1, :1])
        nc.vector.tensor_copy(
            e2k[:, :].rearrange("p (k o) -> p k o", o=1),
            e2tp[:NBW, :].rearrange("p (k x) -> p k x", x=4)[:, 0:NB, 0:1])
        nc.gpsimd.partition_broadcast(e2rep[:], e2row[:])

    wpsum = wpsum_pool.tile([128, 512], F32, tag="wpsum")
    W_sb = const_pool.tile([128, NCHUNK * B], F32, tag="W_sb")
    Weff = const_pool.tile([128, NCHUNK * B], F32, tag="Weff")
    acc_all = acc_pool.tile([128, XW], BF16, tag="acc_all")

    # ---------------- pipelined prologue pieces + stage-1 --------------------
    def tcasts_quad(c0):
        """PE transposes for chunks c0..c0+3 + one wide cast per tensor."""
        for i, dstT in enumerate((frelT, fwT, fcT)):
            tp = tpsum_pool.tile([128, NHALF], F32, tag="tp")
            for k in range(4):
                c = c0 + k
                ces = slice(c * E, (c + 1) * E)
                nc.tensor.transpose(tp[:100, k * 128 : (k + 1) * 128],
                                    nats[i][:, ces], ident[:])
            if (c0 < 16 and i == 0) or (c0 < 8 and i in (1, 2)):
                nc.vector.tensor_scalar(
                    out=dstT[0:100, c0 * 128 : (c0 + 4) * 128],
                    in0=tp[:100, :], scalar1=1.0, scalar2=None, op0=ALU.mult)
            else:
                nc.scalar.activation(
                    dstT[0:100, c0 * 128 : (c0 + 4) * 128], tp[:100, :],
                    AF.Copy)

    def group_fw(g):
        """f2 + W columns for the 8 chunks of group g."""
        gs = slice(g * GCH * 128, (g + 1) * GCH * 128)  # 1024 f cols
        sqg = sq_pool.tile([100, 3 * 1024], BF16, tag="sqg")
        for i, srcT in enumerate((frelT, fwT, fcT)):
            if g == 0:
                # DVE is idle during the ramp; keep ACT off the critical path
                nc.vector.tensor_tensor(sqg[:, i * 1024 : (i + 1) * 1024],
                                        srcT[0:100, gs], srcT[0:100, gs],
                                        op=ALU.mult)
            else:
                nc.scalar.activation(sqg[:, i * 1024 : (i + 1) * 1024],
                                     srcT[0:100, gs], AF.Square)
        f2st = sq_pool.tile([1, 1024], BF16, tag="f2st")
        for h in range(2):
            f2p = tpsum_pool.tile([128, NHALF], F32, tag="tp")
            for i in range(3):
                nc.tensor.matmul(
                    f2p[:1, 0:512], ones_col[:],
                    sqg[:, i * 1024 + h * 512 : i * 1024 + (h + 1) * 512],
                    start=(i == 0), stop=(i == 2))
            nc.scalar.activation(f2st[:, h * 512 : (h + 1) * 512],
                                 f2p[:1, 0:512], AF.Copy)
        for c in range(g * GCH, (g + 1) * GCH):
            cs = slice(c * 128, (c + 1) * 128)
            ws = slice(c * B, (c + 1) * B)
            lc = c - g * GCH
            nc.tensor.matmul(wpsum[:, ws], frelT[:, cs], relmov[:],
                             start=True, stop=False)
            nc.tensor.matmul(wpsum[:, ws], fwT[0:100, cs], qwmov[0:100, :],
                             start=False, stop=False)
            nc.tensor.matmul(wpsum[:, ws],
                             f2st[0:1, lc * 128 : (lc + 1) * 128],
                             negrow[:], start=False, stop=True)
        gws = slice(g * GCH * B, (g + 1) * GCH * B)
        nc.scalar.activation(W_sb[:, gws], wpsum[:, gws], AF.Copy)
        wv = W_sb[:, gws].rearrange("p (c s) -> p c s", s=B)
        ev = Weff[:, gws].rearrange("p (c s) -> p c s", s=B)
        nc.vector.tensor_tensor(
            ev[:, :, 1:B], wv[:, :, 1:B],
            wv[:, :, 0:1].broadcast_to([128, GCH, B - 1]), op=ALU.subtract)

    def group_fw_half(g, h):
        """f2 + W columns for half h (4 chunks) of group g (ramp only)."""
        c0 = g * GCH + h * 4
        gs = slice(c0 * 128, (c0 + 4) * 128)  # 512 f cols
        sqg = sq_pool.tile([100, 3 * 512], BF16, tag="sqgh")
        for i, srcT in enumerate((frelT, fwT, fcT)):
            nc.vector.tensor_tensor(sqg[:, i * 512 : (i + 1) * 512],
                                    srcT[0:100, gs], srcT[0:100, gs],
                                    op=ALU.mult)
        f2st = sq_pool.tile([1, 512], BF16, tag="f2sth")
        f2p = tpsum_pool.tile([128, NHALF], F32, tag="tp")
        for i in range(3):
            nc.tensor.matmul(f2p[:1, 0:512], ones_col[:],
                             sqg[:, i * 512 : (i + 1) * 512],
                             start=(i == 0), stop=(i == 2))
        nc.scalar.activation(f2st[:], f2p[:1, 0:512], AF.Copy)
        for c in range(c0, c0 + 4):
            cs = slice(c * 128, (c + 1) * 128)
            ws = slice(c * B, (c + 1) * B)
            lc = c - c0
            nc.tensor.matmul(wpsum[:, ws], frelT[:, cs], relmov[:],
                             start=True, stop=False)
            nc.tensor.matmul(wpsum[:, ws], fwT[0:100, cs], qwmov[0:100, :],
                             start=False, stop=False)
            nc.tensor.matmul(wpsum[:, ws],
                             f2st[0:1, lc * 128 : (lc + 1) * 128],
                             negrow[:], start=False, stop=True)
        gws = slice(c0 * B, (c0 + 4) * B)
        nc.scalar.activation(W_sb[:, gws], wpsum[:, gws], AF.Copy)
        wv = W_sb[:, gws].rearrange("p (c s) -> p c s", s=B)
        ev = Weff[:, gws].rearrange("p (c s) -> p c s", s=B)
        nc.vector.tensor_tensor(
            ev[:, :, 1:B], wv[:, :, 1:B],
            wv[:, :, 0:1].broadcast_to([128, 4, B - 1]), op=ALU.subtract)

    # ---------------- tail: per-slot partition reduction ---------------------
    gpred = [None] * NGP
    m5 = fin_pool.tile([NBW, (B - NGP) * NB], F32, tag="m5")

    def _tail_reduce(s):
        sec = slice(s * NN, (s + 1) * NN)
        if s < NGP:
            red = fin_pool.tile([128, NN], BF16, tag=f"gpred{s}")
            nc.gpsimd.partition_all_reduce(red[:], acc_all[:, sec], 128,
                                           ReduceOp.max)
            gpred[s] = red
        else:
            ttp = wpsum_pool.tile([128, NB * 128], BF16, tag="ttp")
            for kk in range(NB):
                bs = slice(s * NN + kk * NBW, s * NN + (kk + 1) * NBW)
                nc.tensor.transpose(ttp[:NBW, kk * 128 : (kk + 1) * 128],
                                    acc_all[:, bs], identb[:])
            mcol = (s - NGP) * NB
            nc.vector.tensor_reduce(
                m5[:, mcol : mcol + NB],
                ttp[:NBW, :].rearrange("p (k x) -> p k x", x=128),
                axis=mybir.AxisListType.X, op=ALU.max)

    def stage1(c, tail=False):
        cs = slice(c * 128, (c + 1) * 128)
        wcol = lambda s: W_sb[:, c * B + s : c * B + s + 1]
        dcol = lambda s: Weff[:, c * B + s : c * B + s + 1]
        cp = cpsum_pool.tile([128, NN], F32, tag="cp")
        nc.tensor.matmul(cp[:, 0:NHALF], fcT[0:100, cs], ent2T[:, 0:NHALF],
                         start=True, stop=True)
        # second half is 488 cols: bytes 2048..4000 stay inside PSUM bank 1
        nc.tensor.matmul(cp[:, NHALF:NN], fcT[0:100, cs], ent2T[:, NHALF:NN],
                         start=True, stop=True)
        xt = acc_all if c == 0 else xall_pool.tile([128, XW], BF16, tag="xall")
        xsec = lambda s: xt[:, s * NN : (s + 1) * NN]
        carrier = xsec(0)
        nc.scalar.activation(carrier, cp[:], AF.Identity, bias=wcol(0))
        extra = (4,) if c in SWAP_ACT_CHUNKS else ()
        for s in tuple(act_slots) + extra:
            nc.scalar.activation(xsec(s), cp[:], AF.Identity, bias=wcol(s))
        for s in (x for x in dve_slots if x not in extra):
            nc.vector.tensor_scalar(
                out=xsec(s), in0=carrier, scalar1=dcol(s), scalar2=None,
                op0=ALU.add)
        if c == 0:
            return
        if not tail:
            nc.vector.tensor_tensor(acc_all[:], acc_all[:], xt[:], op=ALU.max)
        else:
            for s in range(B):
                sec = slice(s * NN, (s + 1) * NN)
                nc.vector.tensor_tensor(acc_all[:, sec], acc_all[:, sec],
                                        xt[:, sec], op=ALU.max)
                _tail_reduce(s)

    # ---------------- main loop ----------------------------------------------
    tcasts_quad(0)
    group_fw_half(0, 0)
    stage1(0)
    stage1(1)
    tcasts_quad(4)
    group_fw_half(0, 1)
    stage1(2)
    stage1(3)
    for g in range(GROUPS):
        for ci in range(4 if g == 0 else 0, GCH):
            c = g * GCH + ci
            stage1(c, tail=(c == NCHUNK - 1))
            if c == 18:
                emit_e2()
            if g + 1 < GROUPS:
                if ci == (4 if g == 0 else 1):
                    tcasts_quad((g + 1) * GCH)
                elif ci == (5 if g == 0 else 4):
                    tcasts_quad((g + 1) * GCH + 4)
                elif ci == GCH - 1:
                    group_fw(g + 1)

    # ---------------- finals -------------------------------------------------
    # gpsimd-path slots (0..NGP-1): row layout [NGP, NN]
    mh = fin_pool.tile([NGP, NN], BF16, tag="mh")
    for s in range(NGP):
        eng = nc.sync if s % 2 == 0 else nc.scalar
        eng.dma_start(mh[s : s + 1, :], gpred[s][0:1, :])
    subh = fin_pool.tile([NGP, NN], F32, tag="subh")
    nc.vector.tensor_tensor(subh[:], e2rep[:], mh[:], op=ALU.subtract)
    nc.scalar.activation(subh[:], subh[:], AF.Exp, scale=-0.5)
    nc.vector.tensor_scalar(
        out=subh[:], in0=subh[:], scalar1=1.0, scalar2=None, op0=ALU.min)
    nc.sync.dma_start(out.ap()[0:NGP, 0:NN], subh[:])

    # transpose-path slots (NGP..B-1): m5 [NBW, (B-NGP)*NB], n = k*NBW + p
    fin5 = fin_pool.tile([NBW, (B - NGP) * NB], F32, tag="fin5")
    for j in range(B - NGP):
        js = slice(j * NB, (j + 1) * NB)
        nc.vector.tensor_tensor(fin5[:, js], e2k[:, :], m5[:, js],
                                op=ALU.subtract)
    nc.scalar.activation(fin5[:], fin5[:], AF.Exp, scale=-0.5)
    nc.vector.tensor_scalar(
        out=fin5[:], in0=fin5[:], scalar1=1.0, scalar2=None, op0=ALU.min)
    for j in range(B - NGP):
        eng = nc.sync if j % 2 == 0 else nc.scalar
        eng.dma_start(
            out.ap()[NGP + j : NGP + j + 1, 0:NN].rearrange(
                "o (k p) -> (o p) k", p=NBW),
            fin5[:, j * NB : (j + 1) * NB],
        )


_NC_CACHE = None


def get_nc():
    global _NC_CACHE
    if _NC_CACHE is None:
        _NC_CACHE = build_bass()
    return _NC_CACHE


def make_in_maps(rel, arg1, arg2, fact_rel, fact_arg1, fact_arg2, entity_embeddings):
    n_per = F // 4

    def pad_fact(m):
        o = np.full((FP, E), 10.0, dtype=np.float32)
        o[:F] = m
        return o

    frp = pad_fact(fact_rel)
    f1p = pad_fact(fact_arg1)
    f2p = pad_fact(fact_arg2)
    relc = np.ascontiguousarray(rel, dtype=np.float32)
    a1c = np.ascontiguousarray(arg1, dtype=np.float32)
    a2c = np.ascontiguousarray(arg2, dtype=np.float32)
    in_maps = []
    for core in range(8):
        score, slot = (0, core) if core < 4 else (1, core - 4)
        ent_pad = np.zeros((1024, E), dtype=np.float32)
        ent_pad[:n_per] = entity_embeddings[slot * n_per : (slot + 1) * n_per]
        if score == 0:
            fc_, fw_, qw_ = f2p, f1p, a1c
        else:
            fc_, fw_, qw_ = f1p, f2p, a2c
        in_maps.append(
            {"f_c": fc_, "f_w": fw_, "f_rel": frp, "ent": ent_pad,
             "rel": relc, "qw": qw_}
        )
    return in_maps


def assemble(results):
    n_per = F // 4
    sp = np.concatenate([results[i]["out"][:, :n_per] for i in range(4)], axis=1)
    po = np.concatenate([results[i]["out"][:, :n_per] for i in range(4, 8)], axis=1)
    return sp.copy(), po.copy()


def kernel(rel, arg1, arg2, fact_rel, fact_arg1, fact_arg2, entity_embeddings):
    nc = get_nc()
    in_maps = make_in_maps(
        rel, arg1, arg2, fact_rel, fact_arg1, fact_arg2, entity_embeddings
    )
    res = bass_utils.run_bass_kernel_spmd(nc, in_maps, core_ids=list(range(8)))
    return assemble(res.results)


# revision 28
# speedup vs baseline: 1.2449x; 1.2100x over previous
"""NeuralKB retrieval kernel v7.

Sharding: 8 cores = 2 scores x 4 entity-quarters; per core B=8 batch slots,
NN=1000 entities, F=4000 facts padded to 4096 (32 chunks of 128 facts on
partitions).

Steady state per chunk: PE matmuls cp = 2*ent@fc into PSUM (halves split
512/488 so neither output straddles a 2KB PSUM bank); ACT produces the
slot-0 carrier + slots 1-3 (activation bias-add straight from PSUM, bias =
W column), DVE derives slots 4-7 from the carrier via tensor_scalar (4x
mode), and folds the chunk into the accumulator with one [128, 8000] bf16
tensor_tensor max (2x mode). On SWAP_ACT_CHUNKS (no prologue pieces) slot 4
moves to ACT to balance the engines (sim: DVE 187us / ACT 185us busy).

W prologue: W[f,b] = 2 rel.frel + 2 qw.fw - |q|^2 - f2 via three PE matmuls
per chunk (f2 enters as a K=1 matmul of the f2 row against -1s, avoiding an
SBUF->SBUF DMA into a fact-row). Group 0 is computed in half-group pieces
interleaved with the first chunks so stage-1 starts ~10us in; groups 1-3
pipeline inside the steady state as in v4.

Tail: the last chunk's max runs per-slot; slots 0-3 partition-reduce on
gpsimd (partition_all_reduce, staggered), slots 4-7 via 8 PE transposes
(bf16 -> bf16 PSUM) + one DVE free-dim tensor_reduce each. Finals run in
both layouts and DMA directly to the output rows.
"""

import contextlib

import numpy as np

import concourse.bass as bass
import concourse.tile as tile
from concourse import bacc, mybir
from concourse import bass_utils
from concourse.masks import make_identity
from concourse.bass_isa import ReduceOp

F32 = mybir.dt.float32
BF16 = mybir.dt.bfloat16
AF = mybir.ActivationFunctionType
ALU = mybir.AluOpType

B = 8
E = 100
F = 4000
FP = 4096
NCHUNK = FP // 128
GROUPS = 4
GCH = NCHUNK // GROUPS  # 8
NN = 1000
NHALF = 512          # PSUM-bank-aligned first half; second half is 488 cols
XW = B * NN

DVE_SLOTS = (4, 5, 6, 7)
ACT_SLOTS = (1, 2, 3)

NGP = 4          # tail slots reduced on gpsimd; the rest via PE transposes
# chunks where slot 4 is produced by ACT instead of DVE (DVE is the pole;
# these chunks carry no prologue pieces so ACT has local slack)
SWAP_ACT_CHUNKS = (10, 13, 16, 21, 24, 26, 28)
# ramp chunks where ACT does only the carrier and the (then idle) DVE
# derives all other slots, so the pipeline fills at DVE pace
RAMP_DVE_CHUNKS = 2
NB = 8           # transpose blocks per slot
NBW = NN // NB   # 125


def build_bass(repeat=1, dve_slots=DVE_SLOTS, act_slots=ACT_SLOTS,
               debug=False, scope="full"):
    assert sorted((0,) + tuple(dve_slots) + tuple(act_slots)) == list(range(B))
    nc = bacc.Bacc("TRN2", target_bir_lowering=False, debug=False, num_devices=8)

    f_c = nc.dram_tensor("f_c", [FP, E], F32, kind="ExternalInput")
    f_w = nc.dram_tensor("f_w", [FP, E], F32, kind="ExternalInput")
    f_rel = nc.dram_tensor("f_rel", [FP, E], F32, kind="ExternalInput")
    ent = nc.dram_tensor("ent", [1024, E], F32, kind="ExternalInput")
    rel = nc.dram_tensor("rel", [B, E], F32, kind="ExternalInput")
    qw = nc.dram_tensor("qw", [B, E], F32, kind="ExternalInput")
    out = nc.dram_tensor("out", [B, 1024], F32, kind="ExternalOutput")

    with tile.TileContext(nc) as tc:
        with (
            tc.tile_pool(name="const", bufs=1) as const_pool,
            tc.tile_pool(name="factT", bufs=1) as factT_pool,
            tc.tile_pool(name="acc", bufs=1) as acc_pool,
            tc.tile_pool(name="small", bufs=1) as small_pool,
            tc.tile_pool(name="nat", bufs=1) as nat_pool,
            tc.tile_pool(name="sq", bufs=2) as sq_pool,
            tc.tile_pool(name="xall", bufs=4) as xall_pool,
            tc.tile_pool(name="fin", bufs=1) as fin_pool,
            tc.tile_pool(name="tpsum", bufs=2, space="PSUM") as tpsum_pool,
            tc.tile_pool(name="cpsum", bufs=2, space="PSUM") as cpsum_pool,
            tc.tile_pool(name="wpsum", bufs=1, space="PSUM") as wpsum_pool,
        ):
            pools = (const_pool, factT_pool, acc_pool, small_pool, nat_pool,
                     sq_pool, xall_pool, fin_pool, tpsum_pool, cpsum_pool,
                     wpsum_pool)

            ident = const_pool.tile([128, 128], F32, tag="ident")
            make_identity(nc, ident[:])
            identb = const_pool.tile([128, 128], BF16, tag="identb")
            nc.scalar.activation(identb[:], ident[:], AF.Copy)
            frelT = factT_pool.tile([101, FP], BF16, tag="frelT")
            fwT = factT_pool.tile([101, FP], BF16, tag="fwT")
            fcT = factT_pool.tile([100, FP], BF16, tag="fcT")
            onesrow = small_pool.tile([1, FP], BF16, tag="onesrow")
            nc.gpsimd.memset(onesrow[:], 1.0)
            nc.sync.dma_start(frelT[100:101, :], onesrow[:])
            relmov = const_pool.tile([101, B], BF16, tag="relmov")
            qwmov = const_pool.tile([101, B], BF16, tag="qwmov")
            negrow = small_pool.tile([1, B], BF16, tag="negrow")
            nc.gpsimd.memset(negrow[:], -1.0)
            ones_col = const_pool.tile([100, 1], BF16, tag="ones_col")
            nc.gpsimd.memset(ones_col[:], 1.0)
            statics = (ident, identb, frelT, fwT, fcT, relmov, qwmov, ones_col, negrow)

            rep_ctx = tc.For_i(0, repeat, 1) if repeat > 1 else contextlib.nullcontext()
            with rep_ctx:
                _full_body(nc, tc, pools, statics, f_c, f_w, f_rel, ent,
                           rel, qw, out, dve_slots, act_slots)
    nc.compile()
    return nc


def _full_body(nc, tc, pools, statics, f_c, f_w, f_rel, ent, rel, qw, out,
               dve_slots, act_slots):
    (const_pool, factT_pool, acc_pool, small_pool, nat_pool, sq_pool,
     xall_pool, fin_pool, tpsum_pool, cpsum_pool, wpsum_pool) = pools
    (ident, identb, frelT, fwT, fcT, relmov, qwmov, ones_col, negrow) = statics

    # ---------------- input loads (parallel DGE queues) ----------------------
    nats = []
    for i, dram in enumerate((f_rel, f_w, f_c)):
        natt = nat_pool.tile([128, NCHUNK * E], F32, tag=f"nat_{i}")
        nats.append(natt)

    def load_chunks(c0, c1, engines):
        for i, dram in enumerate((f_rel, f_w, f_c)):
            engines[i].dma_start(
                nats[i][:, c0 * E : c1 * E].rearrange("p (c e) -> p c e", e=E),
                dram.ap()[c0 * 128 : c1 * 128, :].rearrange(
                    "(c p) e -> p c e", p=128
                ),
            )

    qts = {}
    for name, dram in (("rel", rel), ("qw", qw)):
        qt = small_pool.tile([B, E], F32, tag=f"q_{name}")
        nc.sync.dma_start(qt[:], dram.ap())
        qts[name] = qt
    # group-0 chunks first (the W/f2 prologue and first carriers need them,
    # and their consumer chain is longer than the entity path's);
    # alternate the two HWDGE queues per batch to balance bytes.
    load_chunks(0, 4, (nc.sync, nc.scalar, nc.sync))
    entn = nat_pool.tile([128, 8 * E], F32, tag="entn")
    nc.sync.dma_start(
        entn[:].rearrange("p (c e) -> p c e", e=E),
        ent.ap().rearrange("(c p) e -> p c e", p=128),
    )
    load_chunks(4, 8, (nc.scalar, nc.sync, nc.scalar))
    load_chunks(8, 20, (nc.scalar, nc.sync, nc.scalar))
    load_chunks(20, 32, (nc.sync, nc.scalar, nc.sync))

    # ---------------- queries ------------------------------------------------
    for name, dst in (("rel", relmov), ("qw", qwmov)):
        tp = tpsum_pool.tile([128, NHALF], F32, tag="tp")
        nc.tensor.transpose(tp[:E, :B], qts[name][:], ident[:B, :B])
        nc.scalar.activation(dst[0:100, :], tp[:E, :B], AF.Copy, scale=2.0)
    sqs = {}
    for name in ("rel", "qw"):
        sq = small_pool.tile([B, E], F32, tag=f"qsq_{name}")
        nc.scalar.activation(sq[:], qts[name][:], AF.Square)
        r = small_pool.tile([B, 1], F32, tag=f"qr_{name}")
        nc.vector.tensor_reduce(r[:], sq[:], axis=mybir.AxisListType.X, op=ALU.add)
        sqs[name] = r
    q2 = small_pool.tile([B, 1], F32, tag="q2")
    nc.vector.tensor_tensor(q2[:], sqs["rel"][:], sqs["qw"][:], op=ALU.add)
    q2tp = tpsum_pool.tile([128, NHALF], F32, tag="tp")
    nc.tensor.transpose(q2tp[:1, :B], q2[:], ident[:B, :B])
    q2neg = small_pool.tile([1, B], BF16, tag="q2neg")
    nc.scalar.activation(q2neg[:], q2tp[:1, :B], AF.Copy, scale=-1.0)
    nc.sync.dma_start(relmov[100:101, :], q2neg[:])

    # ---------------- entities ----------------------------------------------
    ent2T = const_pool.tile([100, 1024], BF16, tag="ent2T")
    for c in range(8):
        tp = tpsum_pool.tile([128, NHALF], F32, tag="tp")
        nc.tensor.transpose(tp[:100, :128], entn[:, c * E : (c + 1) * E], ident[:])
        nc.vector.tensor_scalar(
            out=ent2T[:, c * 128 : (c + 1) * 128], in0=tp[:100, :128],
            scalar1=2.0, scalar2=None, op0=ALU.mult)
    e2row = small_pool.tile([1, NN], F32, tag="e2row")
    e2k = fin_pool.tile([NBW, NB], F32, tag="e2k")
    e2rep = fin_pool.tile([NGP, NN], F32, tag="e2rep")

    def emit_e2():
        entsqT = sq_pool.tile([100, NN], BF16, tag="entsqT")
        nc.vector.tensor_tensor(entsqT[:], ent2T[:, 0:NN], ent2T[:, 0:NN],
                                op=ALU.mult)
        for h0, h1 in ((0, NHALF), (NHALF, NN)):
            e2p = tpsum_pool.tile([128, NHALF], F32, tag="tp")
            nc.tensor.matmul(e2p[:1, 0 : h1 - h0], ones_col[:],
                             entsqT[:, h0:h1], start=True, stop=True)
            nc.scalar.activation(e2row[:, h0:h1], e2p[:1, 0 : h1 - h0],
                                 AF.Copy, scale=0.25)
        # e2 in [NBW, NB] layout (n = k*NBW + p) for the transpose-path finals
        e2tp = tpsum_pool.tile([128, NHALF], F32, tag="tp")
        for k in range(NB):
            nc.tensor.transpose(e2tp[:NBW, k * 4 : k * 4 + 1],
                                e2row[0:1, k * NBW : (k + 1) * NBW],
                                ident[:1, :1])
        nc.vector.tensor_copy(
            e2k[:, :].rearrange("p (k o) -> p k o", o=1),
            e2tp[:NBW, :].rearrange("p (k x) -> p k x", x=4)[:, 0:NB, 0:1])
        nc.gpsimd.partition_broadcast(e2rep[:], e2row[:])

    wpsum = wpsum_pool.tile([128, 512], F32, tag="wpsum")
    W_sb = const_pool.tile([128, NCHUNK * B], F32, tag="W_sb")
    Weff = const_pool.tile([128, NCHUNK * B], F32, tag="Weff")
    acc_all = acc_pool.tile([128, XW], BF16, tag="acc_all")

    # ---------------- pipelined prologue pieces + stage-1 --------------------
    def tcasts_quad(c0):
        """PE transposes for chunks c0..c0+3 + one wide cast per tensor."""
        for i, dstT in enumerate((frelT, fwT, fcT)):
            tp = tpsum_pool.tile([128, NHALF], F32, tag="tp")
            for k in range(4):
                c = c0 + k
                ces = slice(c * E, (c + 1) * E)
                nc.tensor.transpose(tp[:100, k * 128 : (k + 1) * 128],
                                    nats[i][:, ces], ident[:])
            if (c0 < 16 and i == 0) or (c0 < 8 and i in (1, 2)):
                nc.vector.tensor_scalar(
                    out=dstT[0:100, c0 * 128 : (c0 + 4) * 128],
                    in0=tp[:100, :], scalar1=1.0, scalar2=None, op0=ALU.mult)
            else:
                nc.scalar.activation(
                    dstT[0:100, c0 * 128 : (c0 + 4) * 128], tp[:100, :],
                    AF.Copy)

    def group_fw(g):
        """f2 + W columns for the 8 chunks of group g."""
        gs = slice(g * GCH * 128, (g + 1) * GCH * 128)  # 1024 f cols
        sqg = sq_pool.tile([100, 3 * 1024], BF16, tag="sqg")
        for i, srcT in enumerate((frelT, fwT, fcT)):
            if g == 0:
                # DVE is idle during the ramp; keep ACT off the critical path
                nc.vector.tensor_tensor(sqg[:, i * 1024 : (i + 1) * 1024],
                                        srcT[0:100, gs], srcT[0:100, gs],
                                        op=ALU.mult)
            else:
                nc.scalar.activation(sqg[:, i * 1024 : (i + 1) * 1024],
                                     srcT[0:100, gs], AF.Square)
        f2st = sq_pool.tile([1, 1024], BF16, tag="f2st")
        for h in range(2):
            f2p = tpsum_pool.tile([128, NHALF], F32, tag="tp")
            for i in range(3):
                nc.tensor.matmul(
                    f2p[:1, 0:512], ones_col[:],
                    sqg[:, i * 1024 + h * 512 : i * 1024 + (h + 1) * 512],
                    start=(i == 0), stop=(i == 2))
            nc.scalar.activation(f2st[:, h * 512 : (h + 1) * 512],
                                 f2p[:1, 0:512], AF.Copy)
        for c in range(g * GCH, (g + 1) * GCH):
            cs = slice(c * 128, (c + 1) * 128)
            ws = slice(c * B, (c + 1) * B)
            lc = c - g * GCH
            nc.tensor.matmul(wpsum[:, ws], frelT[:, cs], relmov[:],
                             start=True, stop=False)
            nc.tensor.matmul(wpsum[:, ws], fwT[0:100, cs], qwmov[0:100, :],
                             start=False, stop=False)
            nc.tensor.matmul(wpsum[:, ws],
                             f2st[0:1, lc * 128 : (lc + 1) * 128],
                             negrow[:], start=False, stop=True)
        gws = slice(g * GCH * B, (g + 1) * GCH * B)
        nc.scalar.activation(W_sb[:, gws], wpsum[:, gws], AF.Copy)
        wv = W_sb[:, gws].rearrange("p (c s) -> p c s", s=B)
        ev = Weff[:, gws].rearrange("p (c s) -> p c s", s=B)
        nc.vector.tensor_tensor(
            ev[:, :, 1:B], wv[:, :, 1:B],
            wv[:, :, 0:1].broadcast_to([128, GCH, B - 1]), op=ALU.subtract)

    def group_fw_half(g, h):
        """f2 + W columns for half h (4 chunks) of group g (ramp only)."""
        c0 = g * GCH + h * 4
        gs = slice(c0 * 128, (c0 + 4) * 128)  # 512 f cols
        sqg = sq_pool.tile([100, 3 * 512], BF16, tag="sqgh")
        for i, srcT in enumerate((frelT, fwT, fcT)):
            nc.vector.tensor_tensor(sqg[:, i * 512 : (i + 1) * 512],
                                    srcT[0:100, gs], srcT[0:100, gs],
                                    op=ALU.mult)
        f2st = sq_pool.tile([1, 512], BF16, tag="f2sth")
        f2p = tpsum_pool.tile([128, NHALF], F32, tag="tp")
        for i in range(3):
            nc.tensor.matmul(f2p[:1, 0:512], ones_col[:],
                             sqg[:, i * 512 : (i + 1) * 512],
                             start=(i == 0), stop=(i == 2))
        nc.scalar.activation(f2st[:], f2p[:1, 0:512], AF.Copy)
        for c in range(c0, c0 + 4):
            cs = slice(c * 128, (c + 1) * 128)
            ws = slice(c * B, (c + 1) * B)
            lc = c - c0
            nc.tensor.matmul(wpsum[:, ws], frelT[:, cs], relmov[:],
                             start=True, stop=False)
            nc.tensor.matmul(wpsum[:, ws], fwT[0:100, cs], qwmov[0:100, :],
                             start=False, stop=False)
            nc.tensor.matmul(wpsum[:, ws],
                             f2st[0:1, lc * 128 : (lc + 1) * 128],
                             negrow[:], start=False, stop=True)
        gws = slice(c0 * B, (c0 + 4) * B)
        nc.scalar.activation(W_sb[:, gws], wpsum[:, gws], AF.Copy)
        wv = W_sb[:, gws].rearrange("p (c s) -> p c s", s=B)
        ev = Weff[:, gws].rearrange("p (c s) -> p c s", s=B)
        nc.vector.tensor_tensor(
            ev[:, :, 1:B], wv[:, :, 1:B],
            wv[:, :, 0:1].broadcast_to([128, 4, B - 1]), op=ALU.subtract)

    # ---------------- tail: per-slot partition reduction ---------------------
    gpred = [None] * NGP
    m5 = fin_pool.tile([NBW, (B - NGP) * NB], F32, tag="m5")

    def _tail_reduce(s):
        sec = slice(s * NN, (s + 1) * NN)
        if s < NGP:
            red = fin_pool.tile([128, NN], BF16, tag=f"gpred{s}")
            nc.gpsimd.partition_all_reduce(red[:], acc_all[:, sec], 128,
                                           ReduceOp.max)
            gpred[s] = red
        else:
            ttp = wpsum_pool.tile([128, NB * 128], BF16, tag="ttp")
            for kk in range(NB):
                bs = slice(s * NN + kk * NBW, s * NN + (kk + 1) * NBW)
                nc.tensor.transpose(ttp[:NBW, kk * 128 : (kk + 1) * 128],
                                    acc_all[:, bs], identb[:])
            mcol = (s - NGP) * NB
            nc.vector.tensor_reduce(
                m5[:, mcol : mcol + NB],
                ttp[:NBW, :].rearrange("p (k x) -> p k x", x=128),
                axis=mybir.AxisListType.X, op=ALU.max)

    def stage1(c, tail=False):
        cs = slice(c * 128, (c + 1) * 128)
        wcol = lambda s: W_sb[:, c * B + s : c * B + s + 1]
        dcol = lambda s: Weff[:, c * B + s : c * B + s + 1]
        cp = cpsum_pool.tile([128, NN], F32, tag="cp")
        nc.tensor.matmul(cp[:, 0:NHALF], fcT[0:100, cs], ent2T[:, 0:NHALF],
                         start=True, stop=True)
        # second half is 488 cols: bytes 2048..4000 stay inside PSUM bank 1
        nc.tensor.matmul(cp[:, NHALF:NN], fcT[0:100, cs], ent2T[:, NHALF:NN],
                         start=True, stop=True)
        xt = acc_all if c == 0 else xall_pool.tile([128, XW], BF16, tag="xall")
        xsec = lambda s: xt[:, s * NN : (s + 1) * NN]
        carrier = xsec(0)
        nc.scalar.activation(carrier, cp[:], AF.Identity, bias=wcol(0))
        if c < RAMP_DVE_CHUNKS:
            a_slots, d_slots = (), tuple(range(1, B))
        else:
            extra = (4,) if c in SWAP_ACT_CHUNKS else ()
            a_slots = tuple(act_slots) + extra
            d_slots = tuple(x for x in dve_slots if x not in extra)
        for s in a_slots:
            nc.scalar.activation(xsec(s), cp[:], AF.Identity, bias=wcol(s))
        for s in d_slots:
            nc.vector.tensor_scalar(
                out=xsec(s), in0=carrier, scalar1=dcol(s), scalar2=None,
                op0=ALU.add)
        if c == 0:
            return
        if not tail:
            nc.vector.tensor_tensor(acc_all[:], acc_all[:], xt[:], op=ALU.max)
        else:
            for s in range(B):
                sec = slice(s * NN, (s + 1) * NN)
                nc.vector.tensor_tensor(acc_all[:, sec], acc_all[:, sec],
                                        xt[:, sec], op=ALU.max)
                _tail_reduce(s)

    # ---------------- main loop ----------------------------------------------
    tcasts_quad(0)
    group_fw_half(0, 0)
    stage1(0)
    stage1(1)
    tcasts_quad(4)
    group_fw_half(0, 1)
    stage1(2)
    stage1(3)
    for g in range(GROUPS):
        for ci in range(4 if g == 0 else 0, GCH):
            c = g * GCH + ci
            stage1(c, tail=(c == NCHUNK - 1))
            if c == 18:
                emit_e2()
            if g + 1 < GROUPS:
                if ci == (4 if g == 0 else 1):
                    tcasts_quad((g + 1) * GCH)
                elif ci == (5 if g == 0 else 4):
                    tcasts_quad((g + 1) * GCH + 4)
                elif ci == GCH - 1:
                    group_fw(g + 1)

    # ---------------- finals -------------------------------------------------
    # gpsimd-path slots (0..NGP-1): row layout [NGP, NN]
    mh = fin_pool.tile([NGP, NN], BF16, tag="mh")
    for s in range(NGP):
        eng = nc.sync if s % 2 == 0 else nc.scalar
        eng.dma_start(mh[s : s + 1, :], gpred[s][0:1, :])
    subh = fin_pool.tile([NGP, NN], F32, tag="subh")
    nc.vector.tensor_tensor(subh[:], e2rep[:], mh[:], op=ALU.subtract)
    nc.scalar.activation(subh[:], subh[:], AF.Exp, scale=-0.5)
    nc.vector.tensor_scalar(
        out=subh[:], in0=subh[:], scalar1=1.0, scalar2=None, op0=ALU.min)
    nc.sync.dma_start(out.ap()[0:NGP, 0:NN], subh[:])

    # transpose-path slots (NGP..B-1): m5 [NBW, (B-NGP)*NB], n = k*NBW + p
    fin5 = fin_pool.tile([NBW, (B - NGP) * NB], F32, tag="fin5")
    for j in range(B - NGP):
        js = slice(j * NB, (j + 1) * NB)
        nc.vector.tensor_tensor(fin5[:, js], e2k[:, :], m5[:, js],
                                op=ALU.subtract)
    nc.scalar.activation(fin5[:], fin5[:], AF.Exp, scale=-0.5)
    nc.vector.tensor_scalar(
        out=fin5[:], in0=fin5[:], scalar1=1.0, scalar2=None, op0=ALU.min)
    for j in range(B - NGP):
        eng = nc.sync if j % 2 == 0 else nc.scalar
        eng.dma_start(
            out.ap()[NGP + j : NGP + j + 1, 0:NN].rearrange(
                "o (k p) -> (o p) k", p=NBW),
            fin5[:, j * NB : (j + 1) * NB],
        )


_NC_CACHE = None


def get_nc():
    global _NC_CACHE
    if _NC_CACHE is None:
        _NC_CACHE = build_bass()
    return _NC_CACHE


def make_in_maps(rel, arg1, arg2, fact_rel, fact_arg1, fact_arg2, entity_embeddings):
    n_per = F // 4

    def pad_fact(m):
        o = np.full((FP, E), 10.0, dtype=np.float32)
        o[:F] = m
        return o

    frp = pad_fact(fact_rel)
    f1p = pad_fact(fact_arg1)
    f2p = pad_fact(fact_arg2)
    relc = np.ascontiguousarray(rel, dtype=np.float32)
    a1c = np.ascontiguousarray(arg1, dtype=np.float32)
    a2c = np.ascontiguousarray(arg2, dtype=np.float32)
    in_maps = []
    for core in range(8):
        score, slot = (0, core) if core < 4 else (1, core - 4)
        ent_pad = np.zeros((1024, E), dtype=np.float32)
        ent_pad[:n_per] = entity_embeddings[slot * n_per : (slot + 1) * n_per]
        if score == 0:
            fc_, fw_, qw_ = f2p, f1p, a1c
        else:
            fc_, fw_, qw_ = f1p, f2p, a2c
        in_maps.append(
            {"f_c": fc_, "f_w": fw_, "f_rel": frp, "ent": ent_pad,
             "rel": relc, "qw": qw_}
        )
    return in_maps


def assemble(results):
    n_per = F // 4
    sp = np.concatenate([results[i]["out"][:, :n_per] for i in range(4)], axis=1)
    po = np.concatenate([results[i]["out"][:, :n_per] for i in range(4, 8)], axis=1)
    return sp.copy(), po.copy()


def kernel(rel, arg1, arg2, fact_rel, fact_arg1, fact_arg2, entity_embeddings):
    nc = get_nc()
    in_maps = make_in_maps(
        rel, arg1, arg2, fact_rel, fact_arg1, fact_arg2, entity_embeddings
    )
    res = bass_utils.run_bass_kernel_spmd(nc, in_maps, core_ids=list(range(8)))
    return assemble(res.results)
